# revision 32
# baseline (speedup 1.0000x reference)
"""Trainium2 Bass kernel for nn_Model_11888469475981 (pooling).

Reference semantics (per sample n, channel c, row (d,h) along W):
  pad W by (1,2) -> row A[0..258]; K=3 S=2 maxpool w/ indices (L=129
  windows), softsign, max-unpool scatter, add padded input, mean over
  padded D (17 slabs, one all-zero).

Restructure (per padded row, half-grid m with ev[m]=A[2m], od[m]=A[2m+1]):
  Q[m]   = max(ev[m], od[m], ev[m+1])          window max, m=0..128
  R      = [G | Q] with G[m] = min(Q[m], Q[m-1])  (guards +BIG)
  mask   = A >= R      (one full-width compare: modd | meven)
  SS     = [SG | SQ]:  SQ = softsign(Q), SG = min(SQ, SQ[m-1])
  ms     = mask * SS   (selected positions get softsign(window max))
  out    = (1/17) * sum_d (A + ms)
Masks/values in bf16 (L2 err ~7e-3, gate 2e-2). softsign(Q) is one
custom DVE op (bitwise-NOT reciprocal seed + one Newton step folded
with the numerator; |Q| feeds in from ACT Abs).

Layout per core (1 sample): per 2-channel tile [128, 16*SW]:
  partition p = d*8 + hb (d=0..15, hb=h//8), slot s = ch*8 + h%8
  (channel stride in HBM is exactly 8 slots, so 2 channels form 16
  uniform slots). Parity slot layout (SW=268): cols 0..129 = ev[0..129],
  col 130 guard, 131..259 = od[0..128], 260..267 junk. ACT
  de-interleaves the fp32 DMA tile into this bf16 layout; every DVE op
  is then packed stride-1 (2x bf16 mode). GpSimd is NOT used: its SBUF
  traffic stalls concurrent DVE ops ~2.5x (measured). PE sums over d
  with two chained bf16 matmuls per slot into one 8-bank PSUM tile per
  channel; ACT evacuates with the 1/17 scale folded in, re-interleaving
  parities into natural W order.
"""

import numpy as np
import ml_dtypes

import concourse.bass as bass
import concourse.mybir as mybir
from concourse import bacc
from concourse import dve_ops
from concourse.dve_spec import (
    Spec, Src0, Src1, Zero, One, C0, C1, AluOp as DAlu, Bin, maxx, lower,
)
from concourse.dve_uop import DveOpSpec
from concourse.tile import TileContext
from concourse.bass_utils import run_bass_kernel_spmd

N_CORES = 8
C, D, H, W = 32, 16, 64, 256
HP, WP = 66, 259
NS = 16           # slots per tile: 2 channels x 8 h-subslots
SW = 268          # parity slot: 130 ev | 1 guard | 129 od | 8 junk
GD = 130          # guard col
OD0 = 131
F32 = mybir.dt.float32
BF16 = mybir.dt.bfloat16
Alu = mybir.AluOpType
Act = mybir.ActivationFunctionType
BIG = 1e30
SS_C0, SS_C1 = -0.23549792, 2.0017324   # recip seed/NR consts


def _register_softsign():
    """out = Src1 * y0 * (C1 - d*y0);  d = 1+Src0, y0 = bitcast(~d)*C0.
    Called with in0=|Q|, in1=Q: softsign(Q) via bitwise-NOT reciprocal
    seed + one Newton step folded into the numerator — 7 ALU stages."""
    name = "SOFTSIGN_APPROX_ANT"
    for op in dve_ops.OPS:
        if op.name == name:
            return op
    d = One + Src0
    nd = Bin(DAlu.BITWISE_NOT, d, d)
    y0 = nd * C0
    body = (Src1 * y0) * (C1 - d * y0)

    def _ref(in0, in1, c0, c1, c2):
        dd = (1.0 + in0).astype(np.float32)
        yy = (~dd.view(np.int32)).view(np.float32) * c0
        return (in1 * yy) * (c1 - dd * yy)

    spec = Spec(body=body, reference=_ref)
    row = max(dve_ops._SUB_OPCODE_FOR_NAME.values()) + 1
    assert row < 0x20
    shas = {}
    for ver in ("v3", "v4"):
        try:
            tmp = DveOpSpec(name=name, opcode=row,
                            uops=lower(spec, ver=ver), rd1_en=True)
            shas[ver] = tmp.sha(ver)
        except Exception:
            pass
    op = dve_ops.DveOp(name, spec, subdim=False, uops_sha=shas)
    dve_ops.OPS.append(op)
    dve_ops.CUSTOM_DVE_SPECS[name] = spec
    dve_ops._SUB_OPCODE_FOR_NAME[name] = row
    return op


def _s(t, w):
    return t[:].rearrange("p (s w) -> p s w", w=w)


def _act(nc, out, in_, func, scale=1.0, bias=0.0):
    # direct InstActivation emission (the nc.scalar.activation wrapper
    # refuses Reciprocal; its |err|~4e-3 is fine for softsign values)
    eng = nc.scalar
    ins = [eng.lower_ap(in_)]
    for arg in (bias, scale, 0.0):
        ins.append(mybir.ImmediateValue(dtype=mybir.dt.float32, value=arg))
    return eng.add_instruction(
        mybir.InstActivation(name=nc.get_next_instruction_name(),
                             func=func, ins=ins, outs=[eng.lower_ap(out)]))


def build_nc():
    ss_op = _register_softsign()
    nc = bacc.Bacc()
    x_ext = nc.declare_dram_parameter("x", [C, D, H, W], F32, isOutput=False)
    w8_ext = nc.declare_dram_parameter("w8", [128, 8], BF16, isOutput=False)
    out_ext = nc.declare_dram_parameter("out", [C, HP, WP], F32, isOutput=True)

    with TileContext(nc) as tc:
        with tc.tile_pool(name="main", bufs=1) as pool, \
             tc.tile_pool(name="psum", bufs=1, space="PSUM") as psum_pool:
            def tiles(nm, shape, dtype):
                return [pool.tile(shape, dtype, tag=f"{nm}{i}",
                                  name=f"{nm}{i}") for i in range(2)]

            a_ts = [pool.tile([128, NS * W], F32, tag=f"a{i}",
                              name=f"a{i}") for i in range(3)]
            ab_ts = tiles("ab", [128, NS * SW], BF16)
            p_ts = tiles("p", [128, NS * 136], BF16)
            aq_ts = tiles("aq", [128, NS * 136], BF16)
            r_ts = tiles("r", [128, NS * SW], BF16)
            ss_ts = tiles("ss", [128, NS * SW], BF16)
            mk_ts = tiles("mk", [128, NS * SW], BF16)
            ms_ts = tiles("ms", [128, NS * SW], BF16)
            # slot width 260 (not 259) so stride-2 parity views factorize
            o_ts = tiles("o", [8, NS * 260], F32)
            w8_t = pool.tile([128, 8], BF16, tag="w8", name="w8")
            z_t = pool.tile([32, 2 * WP], F32, tag="zrow", name="zrow")

            # one-time init of the narrow pad/guard/junk column ranges only
            # (real cols are fully rewritten every iteration; pads persist).
            # Abf: W-pads + guard + junk = 0. R: guard + junk = +BIG (serve
            # as Q[-1]/Q[129] guards for G). SS: guard + junk = 1.0
            # (= softsign(BIG), guards for SG). o_t: the always-zero output
            # cols w'=0,257,258 (+ slot pad col).
            for t in ab_ts:
                v = _s(t, SW)
                nc.vector.memset(v[:, :, 0:1], 0.0)
                nc.vector.memset(v[:, :, 129:131], 0.0)
                nc.vector.memset(v[:, :, 259:SW], 0.0)
            for t in r_ts:
                v = _s(t, SW)
                nc.vector.memset(v[:, :, GD:GD + 1], BIG)
                nc.vector.memset(v[:, :, 260:SW], BIG)
            for t in ss_ts:
                v = _s(t, SW)
                nc.vector.memset(v[:, :, GD:GD + 1], 1.0)
                nc.vector.memset(v[:, :, 260:SW], 1.0)
            for t in o_ts:
                v4 = t[:].rearrange("p (s w2 two) -> p s w2 two",
                                    s=NS, two=2)
                nc.vector.memset(v4[:, :, 0:1, 0], 0.0)
                nc.vector.memset(v4[:, :, 129:130, 0], 0.0)
                nc.vector.memset(v4[:, :, 128:130, 1], 0.0)
            nc.vector.memset(z_t[:], 0.0)
            nc.sync.dma_start(out=w8_t[:], in_=w8_ext[:, :])

            # padded-H border rows (h'=0 and h'=65) for every channel: zeros
            nc.sync.dma_start(
                out=bass.AP(out_ext, 0, [[HP * WP, C], [65 * WP, 2], [1, WP]]),
                in_=z_t[:].rearrange("p (a w) -> p a w", w=WP),
            )

            def dma_in(i):
                nc.sync.dma_start(
                    out=a_ts[i % 3][:],
                    in_=bass.AP(x_ext, 2 * i * D * H * W,
                                [[2048, 128], [D * H * W, 2], [1, 2048]]),
                )

            def dei(i):
                # de-interleave + cast on ACT:
                #   ev[m]=x[2m-1] (m=1..128, odd reals)
                #   od[m]=x[2m]   (m=0..127, even reals)
                av2 = a_ts[i % 3][:].rearrange(
                    "p (s w2 two) -> p s w2 two", s=NS, two=2)
                abv = _s(ab_ts[i % 2], SW)
                nc.scalar.copy(abv[:, :, 1:129], av2[:, :, 0:128, 1])
                nc.scalar.copy(abv[:, :, OD0:OD0 + 128], av2[:, :, 0:128, 0])

            dma_in(0)
            dma_in(1)
            dei(0)
            for i in range(C // 2):
                c0, it = 2 * i, i % 2
                ab_t, p_t, aq_t = (t[it] for t in (ab_ts, p_ts, aq_ts))
                r_t, ss_t, mk_t, ms_t, o_t = (t[it] for t in
                                              (r_ts, ss_ts, mk_ts, ms_ts,
                                               o_ts))
                abv = _s(ab_t, SW)

                if i + 2 < C // 2:
                    dma_in(i + 2)

                # A-matmuls, both channels (start; ms-phase stops). PSUM
                # holds only the 256 real W cols per slot (outputs at
                # padded W are always zero, pre-zeroed in o_t), packed as
                # 4 banks/channel: [ev s0-3][ev s4-7][od s0-3][od s4-7],
                # one 512-row matmul per bank -> PE double-buffers.
                pss = []
                for j in range(2):
                    ps = psum_pool.tile([8, 4 * 512], F32, tag=f"ps{j}",
                                        name=f"ps_{c0}_{j}")
                    pss.append(ps)
                    psb = ps[:].rearrange("p (b s w) -> p b s w", b=4, w=128)
                    for g in range(2):
                        nc.tensor.matmul(psb[:, g], w8_t[:, 0:8],
                                         abv[:, 8 * j + 4 * g:
                                             8 * j + 4 * g + 4,
                                             OD0:OD0 + 128],
                                         start=True, stop=False)
                        nc.tensor.matmul(psb[:, 2 + g], w8_t[:, 0:8],
                                         abv[:, 8 * j + 4 * g:
                                             8 * j + 4 * g + 4, 1:129],
                                         start=True, stop=False)

                rv = _s(r_t, SW)
                ev = abv[:, :, 0:129]
                evp = abv[:, :, 1:130]
                od = abv[:, :, OD0:OD0 + 129]
                pv = _s(p_t, 136)[:, :, 0:129]
                q_ = rv[:, :, OD0:OD0 + 129]

                # window max Q -> R_od, neighbor-min G -> R_ev (DVE bf16 2x)
                nc.vector.tensor_tensor(pv, ev, od, Alu.max)
                nc.vector.tensor_tensor(q_, pv, evp, Alu.max)
                nc.vector.tensor_tensor(
                    rv[:, :, 0:130], rv[:, :, OD0:OD0 + 130],
                    rv[:, :, GD:GD + 130], Alu.min)

                # softsign: ACT Abs feeds the fused custom op -> SS_od,
                # then SG = min-shift -> SS_ev
                aqv = _s(aq_t, 136)[:, :, 0:129]
                nc.scalar.activation(aqv, q_, Act.Abs)

                # next iteration's de-interleave is emitted here so the
                # ACT queue never parks it behind this iteration's evac
                if i + 1 < C // 2:
                    dei(i + 1)

                # mask first: overlaps the ACT Abs so the custom softsign
                # op doesn't stall the DVE queue
                mkv = _s(mk_t, SW)
                nc.vector.tensor_tensor(mkv[:, :, 0:260], abv[:, :, 0:260],
                                        rv[:, :, 0:260], Alu.is_ge)

                ssv = _s(ss_t, SW)
                nc.vector._custom_dve(ss_op, out=ssv[:, :, OD0:OD0 + 129],
                                      in0=aqv, in1=q_, s0=SS_C0, s1=SS_C1)
                nc.vector.tensor_tensor(
                    ssv[:, :, 0:130], ssv[:, :, OD0:OD0 + 130],
                    ssv[:, :, GD:GD + 130], Alu.min)
                msv = _s(ms_t, SW)
                nc.vector.tensor_tensor(msv[:, :, 0:260], mkv[:, :, 0:260],
                                        _s(ss_t, SW)[:, :, 0:260], Alu.mult)

                # per channel: ms-matmuls (stop), evacuate (1/17 +
                # re-interleave into pre-zeroed o_t), DMA out
                ov4 = o_t[:].rearrange("p (s w2 two) -> p s w2 two",
                                       s=NS, two=2)
                for j in range(2):
                    psb = pss[j][:].rearrange("p (b s w) -> p b s w",
                                              b=4, w=128)
                    for g in range(2):
                        nc.tensor.matmul(psb[:, g], w8_t[:, 0:8],
                                         msv[:, 8 * j + 4 * g:
                                             8 * j + 4 * g + 4,
                                             OD0:OD0 + 128],
                                         start=False, stop=True)
                        nc.tensor.matmul(psb[:, 2 + g], w8_t[:, 0:8],
                                         msv[:, 8 * j + 4 * g:
                                             8 * j + 4 * g + 4, 1:129],
                                         start=False, stop=True)
                    psf = pss[j][:].rearrange("p (s w) -> p s w", s=16)
                    oh = ov4[:, 8 * j:8 * j + 8]
                    nc.scalar.mul(oh[:, :, 0:128, 1],
                                  psf[:, 0:8, :], 1.0 / 17.0)
                    nc.scalar.mul(oh[:, :, 1:129, 0],
                                  psf[:, 8:16, :], 1.0 / 17.0)
                    nc.sync.dma_start(
                        out=bass.AP(out_ext, ((c0 + j) * HP + 1) * WP,
                                    [[8 * WP, 8], [WP, 8], [1, WP]]),
                        in_=_s(o_t, 260)[:, 8 * j:8 * j + 8, 0:WP],
                    )
    nc.finalize()
    return nc


_CACHE: dict = {}


def _get_nc():
    if "nc" not in _CACHE:
        _CACHE["nc"] = build_nc()
    return _CACHE["nc"]


def make_in_maps(x: np.ndarray):
    w8 = np.zeros((128, 8), ml_dtypes.bfloat16)
    w8[np.arange(128), np.arange(128) % 8] = 1.0
    return [
        {"x": np.ascontiguousarray(x[i]), "w8": w8}
        for i in range(N_CORES)
    ]


def kernel(**inputs) -> np.ndarray:
    x = np.ascontiguousarray(np.asarray(inputs["x"], dtype=np.float32))
    assert x.shape == (N_CORES, C, D, H, W), x.shape
    nc = _get_nc()
    res = run_bass_kernel_spmd(nc, make_in_maps(x), list(range(N_CORES)))
    return np.stack([res.results[i]["out"] for i in range(N_CORES)], axis=0)


# revision 33
# speedup vs baseline: 1.0380x; 1.0380x over previous
"""Trainium2 Bass kernel for nn_Model_11888469475981 (pooling).

Reference semantics (per sample n, channel c, row (d,h) along W):
  pad W by (1,2) -> row A[0..258]; K=3 S=2 maxpool w/ indices (L=129
  windows), softsign, max-unpool scatter, add padded input, mean over
  padded D (17 slabs, one all-zero).

Restructure (per padded row, half-grid m with ev[m]=A[2m], od[m]=A[2m+1]):
  Q[m]   = max(ev[m], od[m], ev[m+1])          window max, m=0..128
  R      = [G | Q] with G[m] = min(Q[m], Q[m-1])  (guards +BIG)
  mask   = A >= R      (one full-width compare: modd | meven)
  SS     = [SG | SQ]:  SQ = softsign(Q), SG = min(SQ, SQ[m-1])
  ms     = mask * SS   (selected positions get softsign(window max))
  out    = (1/17) * sum_d (A + ms)
Masks/values in bf16 (L2 err ~7e-3, gate 2e-2). softsign(Q) is one
custom DVE op (bitwise-NOT reciprocal seed + one Newton step folded
with the numerator; |Q| feeds in from ACT Abs).

Layout per core (1 sample): per 2-channel tile [128, 16*SW]:
  partition p = d*8 + hb (d=0..15, hb=h//8), slot s = ch*8 + h%8
  (channel stride in HBM is exactly 8 slots, so 2 channels form 16
  uniform slots). Parity slot layout (SW=268): cols 0..129 = ev[0..129],
  col 130 guard, 131..259 = od[0..128], 260..267 junk. ACT
  de-interleaves the fp32 DMA tile into this bf16 layout; every DVE op
  is then packed stride-1 (2x bf16 mode). GpSimd is NOT used: its SBUF
  traffic stalls concurrent DVE ops ~2.5x (measured). PE sums over d
  with two chained bf16 matmuls per slot into one 8-bank PSUM tile per
  channel; ACT evacuates with the 1/17 scale folded in, re-interleaving
  parities into natural W order.
"""

import numpy as np
import ml_dtypes

import concourse.bass as bass
import concourse.mybir as mybir
from concourse import bacc
from concourse import dve_ops
from concourse.dve_spec import (
    Spec, Src0, Src1, Zero, One, C0, C1, AluOp as DAlu, Bin, maxx, lower,
)
from concourse.dve_uop import DveOpSpec
from concourse.tile import TileContext
from concourse.bass_utils import run_bass_kernel_spmd

N_CORES = 8
C, D, H, W = 32, 16, 64, 256
HP, WP = 66, 259
NS = 16           # slots per tile: 2 channels x 8 h-subslots
SW = 268          # parity slot: 130 ev | 1 guard | 129 od | 8 junk
GD = 130          # guard col
OD0 = 131
F32 = mybir.dt.float32
BF16 = mybir.dt.bfloat16
Alu = mybir.AluOpType
Act = mybir.ActivationFunctionType
BIG = 1e30
SS_C0, SS_C1 = -0.23549792, 2.0017324   # recip seed/NR consts


def _register_softsign():
    """out = Src1 * y0 * (C1 - d*y0);  d = 1+Src0, y0 = bitcast(~d)*C0.
    Called with in0=|Q|, in1=Q: softsign(Q) via bitwise-NOT reciprocal
    seed + one Newton step folded into the numerator — 7 ALU stages."""
    name = "SOFTSIGN_APPROX_ANT"
    for op in dve_ops.OPS:
        if op.name == name:
            return op
    d = One + Src0
    nd = Bin(DAlu.BITWISE_NOT, d, d)
    y0 = nd * C0
    body = (Src1 * y0) * (C1 - d * y0)

    def _ref(in0, in1, c0, c1, c2):
        dd = (1.0 + in0).astype(np.float32)
        yy = (~dd.view(np.int32)).view(np.float32) * c0
        return (in1 * yy) * (c1 - dd * yy)

    spec = Spec(body=body, reference=_ref)
    row = max(dve_ops._SUB_OPCODE_FOR_NAME.values()) + 1
    assert row < 0x20
    shas = {}
    for ver in ("v3", "v4"):
        try:
            tmp = DveOpSpec(name=name, opcode=row,
                            uops=lower(spec, ver=ver), rd1_en=True)
            shas[ver] = tmp.sha(ver)
        except Exception:
            pass
    op = dve_ops.DveOp(name, spec, subdim=False, uops_sha=shas)
    dve_ops.OPS.append(op)
    dve_ops.CUSTOM_DVE_SPECS[name] = spec
    dve_ops._SUB_OPCODE_FOR_NAME[name] = row
    return op


def _s(t, w):
    return t[:].rearrange("p (s w) -> p s w", w=w)


def _act(nc, out, in_, func, scale=1.0, bias=0.0):
    # direct InstActivation emission (the nc.scalar.activation wrapper
    # refuses Reciprocal; its |err|~4e-3 is fine for softsign values)
    eng = nc.scalar
    ins = [eng.lower_ap(in_)]
    for arg in (bias, scale, 0.0):
        ins.append(mybir.ImmediateValue(dtype=mybir.dt.float32, value=arg))
    return eng.add_instruction(
        mybir.InstActivation(name=nc.get_next_instruction_name(),
                             func=func, ins=ins, outs=[eng.lower_ap(out)]))


def build_nc():
    ss_op = _register_softsign()
    nc = bacc.Bacc()
    x_ext = nc.declare_dram_parameter("x", [C, D, H, W], F32, isOutput=False)
    w8_ext = nc.declare_dram_parameter("w8", [128, 8], BF16, isOutput=False)
    out_ext = nc.declare_dram_parameter("out", [C, HP, WP], F32, isOutput=True)

    with TileContext(nc) as tc:
        with tc.tile_pool(name="main", bufs=1) as pool, \
             tc.tile_pool(name="psum", bufs=1, space="PSUM") as psum_pool:
            def tiles(nm, shape, dtype):
                return [pool.tile(shape, dtype, tag=f"{nm}{i}",
                                  name=f"{nm}{i}") for i in range(2)]

            a_ts = [pool.tile([128, NS * W], F32, tag=f"a{i}",
                              name=f"a{i}") for i in range(3)]
            ab_ts = tiles("ab", [128, NS * SW], BF16)
            p_ts = tiles("p", [128, NS * 136], BF16)
            aq_ts = tiles("aq", [128, NS * 136], BF16)
            r_ts = tiles("r", [128, NS * SW], BF16)
            ss_ts = tiles("ss", [128, NS * SW], BF16)
            mk_ts = tiles("mk", [128, NS * SW], BF16)
            ms_ts = tiles("ms", [128, NS * SW], BF16)
            # slot width 260 (not 259) so stride-2 parity views factorize
            o_ts = tiles("o", [8, NS * 260], F32)
            w8_t = pool.tile([128, 8], BF16, tag="w8", name="w8")
            z_t = pool.tile([32, 2 * WP], F32, tag="zrow", name="zrow")

            # one-time init of the narrow pad/guard/junk column ranges only
            # (real cols are fully rewritten every iteration; pads persist).
            # Abf: W-pads + guard + junk = 0. R: guard + junk = +BIG (serve
            # as Q[-1]/Q[129] guards for G). SS: guard + junk = 1.0
            # (= softsign(BIG), guards for SG). o_t: the always-zero output
            # cols w'=0,257,258 (+ slot pad col).
            for t in ab_ts:
                v = _s(t, SW)
                nc.vector.memset(v[:, :, 0:1], 0.0)
                nc.vector.memset(v[:, :, 129:131], 0.0)
                nc.vector.memset(v[:, :, 259:SW], 0.0)
            for t in r_ts:
                v = _s(t, SW)
                nc.vector.memset(v[:, :, GD:GD + 1], BIG)
                nc.vector.memset(v[:, :, 260:SW], BIG)
            for t in ss_ts:
                v = _s(t, SW)
                nc.vector.memset(v[:, :, GD:GD + 1], 1.0)
                nc.vector.memset(v[:, :, 260:SW], 1.0)
            for t in o_ts:
                v4 = t[:].rearrange("p (s w2 two) -> p s w2 two",
                                    s=NS, two=2)
                nc.vector.memset(v4[:, :, 0:1, 0], 0.0)
                nc.vector.memset(v4[:, :, 129:130, 0], 0.0)
                nc.vector.memset(v4[:, :, 128:130, 1], 0.0)
            nc.vector.memset(z_t[:], 0.0)
            nc.sync.dma_start(out=w8_t[:], in_=w8_ext[:, :])

            # padded-H border rows (h'=0 and h'=65) for every channel: zeros
            nc.sync.dma_start(
                out=bass.AP(out_ext, 0, [[HP * WP, C], [65 * WP, 2], [1, WP]]),
                in_=z_t[:].rearrange("p (a w) -> p a w", w=WP),
            )

            def dma_in(i):
                nc.sync.dma_start(
                    out=a_ts[i % 3][:],
                    in_=bass.AP(x_ext, 2 * i * D * H * W,
                                [[2048, 128], [D * H * W, 2], [1, 2048]]),
                )

            def dei(i):
                # de-interleave + cast on ACT:
                #   ev[m]=x[2m-1] (m=1..128, odd reals)
                #   od[m]=x[2m]   (m=0..127, even reals)
                av2 = a_ts[i % 3][:].rearrange(
                    "p (s w2 two) -> p s w2 two", s=NS, two=2)
                abv = _s(ab_ts[i % 2], SW)
                nc.scalar.copy(abv[:, :, 1:129], av2[:, :, 0:128, 1])
                nc.scalar.copy(abv[:, :, OD0:OD0 + 128], av2[:, :, 0:128, 0])

            dma_in(0)
            dma_in(1)
            dei(0)
            for i in range(C // 2):
                c0, it = 2 * i, i % 2
                ab_t, p_t, aq_t = (t[it] for t in (ab_ts, p_ts, aq_ts))
                r_t, ss_t, mk_t, ms_t, o_t = (t[it] for t in
                                              (r_ts, ss_ts, mk_ts, ms_ts,
                                               o_ts))
                abv = _s(ab_t, SW)

                if i + 2 < C // 2:
                    dma_in(i + 2)

                # A-matmuls, both channels (start; ms-phase stops). PSUM
                # holds only the 256 real W cols per slot (outputs at
                # padded W are always zero, pre-zeroed in o_t), packed as
                # 4 banks/channel: [ev s0-3][ev s4-7][od s0-3][od s4-7],
                # one 512-row matmul per bank -> PE double-buffers.
                pss = []
                for j in range(2):
                    ps = psum_pool.tile([8, 4 * 512], F32, tag=f"ps{j}",
                                        name=f"ps_{c0}_{j}")
                    pss.append(ps)
                    psb = ps[:].rearrange("p (b s w) -> p b s w", b=4, w=128)
                    for g in range(2):
                        nc.tensor.matmul(psb[:, g], w8_t[:, 0:8],
                                         abv[:, 8 * j + 4 * g:
                                             8 * j + 4 * g + 4,
                                             OD0:OD0 + 128],
                                         start=True, stop=False)
                        nc.tensor.matmul(psb[:, 2 + g], w8_t[:, 0:8],
                                         abv[:, 8 * j + 4 * g:
                                             8 * j + 4 * g + 4, 1:129],
                                         start=True, stop=False)

                rv = _s(r_t, SW)
                ev = abv[:, :, 0:129]
                evp = abv[:, :, 1:130]
                od = abv[:, :, OD0:OD0 + 129]
                pv = _s(p_t, 136)[:, :, 0:129]
                q_ = rv[:, :, OD0:OD0 + 129]

                # window max Q -> R_od, neighbor-min G -> R_ev (DVE bf16 2x)
                nc.vector.tensor_tensor(pv, ev, od, Alu.max)
                nc.vector.tensor_tensor(q_, pv, evp, Alu.max)
                nc.vector.tensor_tensor(
                    rv[:, :, 0:130], rv[:, :, OD0:OD0 + 130],
                    rv[:, :, GD:GD + 130], Alu.min)

                # softsign: ACT Abs feeds the fused custom op -> SS_od,
                # then SG = min-shift -> SS_ev
                aqv = _s(aq_t, 136)[:, :, 0:129]
                nc.scalar.activation(aqv, q_, Act.Abs)

                # next iteration's de-interleave is emitted here so the
                # ACT queue never parks it behind this iteration's evac
                if i + 1 < C // 2:
                    dei(i + 1)

                # mask first: overlaps the ACT Abs so the custom softsign
                # op doesn't stall the DVE queue
                mkv = _s(mk_t, SW)
                nc.vector.tensor_tensor(mkv[:, :, 0:260], abv[:, :, 0:260],
                                        rv[:, :, 0:260], Alu.is_ge)

                ssv = _s(ss_t, SW)
                nc.vector._custom_dve(ss_op, out=ssv[:, :, OD0:OD0 + 129],
                                      in0=aqv, in1=q_, s0=SS_C0, s1=SS_C1)
                nc.vector.tensor_tensor(
                    ssv[:, :, 0:130], ssv[:, :, OD0:OD0 + 130],
                    ssv[:, :, GD:GD + 130], Alu.min)
                msv = _s(ms_t, SW)
                nc.vector.tensor_tensor(msv[:, :, 0:260], mkv[:, :, 0:260],
                                        _s(ss_t, SW)[:, :, 0:260], Alu.mult)

                # per channel: ms-matmuls (stop), evacuate (1/17 +
                # re-interleave into pre-zeroed o_t), DMA out
                ov4 = o_t[:].rearrange("p (s w2 two) -> p s w2 two",
                                       s=NS, two=2)
                for j in range(2):
                    psb = pss[j][:].rearrange("p (b s w) -> p b s w",
                                              b=4, w=128)
                    for g in range(2):
                        nc.tensor.matmul(psb[:, g], w8_t[:, 0:8],
                                         msv[:, 8 * j + 4 * g:
                                             8 * j + 4 * g + 4,
                                             OD0:OD0 + 128],
                                         start=False, stop=True)
                        nc.tensor.matmul(psb[:, 2 + g], w8_t[:, 0:8],
                                         msv[:, 8 * j + 4 * g:
                                             8 * j + 4 * g + 4, 1:129],
                                         start=False, stop=True)
                    psm = pss[j][:].rearrange(
                        "p (two s w) -> p s w two", two=2, w=128)
                    oh4 = _s(o_t, 260)[:, 8 * j:8 * j + 8, 1:257].rearrange(
                        "p s (w two) -> p s w two", two=2)
                    nc.scalar.mul(oh4, psm, 1.0 / 17.0)
                    nc.sync.dma_start(
                        out=bass.AP(out_ext, ((c0 + j) * HP + 1) * WP,
                                    [[8 * WP, 8], [WP, 8], [1, WP]]),
                        in_=_s(o_t, 260)[:, 8 * j:8 * j + 8, 0:WP],
                    )
    nc.finalize()
    return nc


_CACHE: dict = {}


def _get_nc():
    if "nc" not in _CACHE:
        _CACHE["nc"] = build_nc()
    return _CACHE["nc"]


def make_in_maps(x: np.ndarray):
    w8 = np.zeros((128, 8), ml_dtypes.bfloat16)
    w8[np.arange(128), np.arange(128) % 8] = 1.0
    return [
        {"x": np.ascontiguousarray(x[i]), "w8": w8}
        for i in range(N_CORES)
    ]


def kernel(**inputs) -> np.ndarray:
    x = np.ascontiguousarray(np.asarray(inputs["x"], dtype=np.float32))
    assert x.shape == (N_CORES, C, D, H, W), x.shape
    nc = _get_nc()
    res = run_bass_kernel_spmd(nc, make_in_maps(x), list(range(N_CORES)))
    return np.stack([res.results[i]["out"] for i in range(N_CORES)], axis=0)
